# revision 13
# baseline (speedup 1.0000x reference)
# Trainium2 Bass kernel for nn_DetectionHead (nms_detection).
#
# Strategy (data-parallel, batch 32 -> 8 cores x 4 images):
#   Device (per core): stream all cls planes, compute d = l1 - l0 (exact f32),
#   hardware top-8-per-partition-half-row via DVE max/max_index ->
#   2048 (value, index) candidates per image.
#   Host: map candidates to global locations, score them with the exact same
#   jax-CPU softmax ops as the reference (bitwise), replicate lax.top_k
#   ordering (desc score, ties by ascending index), decode boxes in exact
#   f32, replicate greedy NMS bitwise, assemble outputs.
#   A provable validation rule triggers an exact full-image numpy/jax
#   fallback in the (astronomically rare) case the candidate set cannot be
#   proven to contain the true top-150.
#
# The candidate set is provably a superset of the reference top-150 when,
# for every half-row, the 8th (smallest) candidate's masked score is
# strictly below the selected 150th score (hidden elements of that half-row
# have d <= the 8th candidate's d, hence score <= its score).
import numpy as np

SIZES = [(256, 256), (256, 256), (128, 128), (128, 128), (64, 64), (32, 32), (32, 32), (32, 32)]
STRIDES = [4, 4, 8, 8, 16, 32, 32, 32]
RF_SIZES = [55.0, 71.0, 111.0, 143.0, 223.0, 383.0, 511.0, 639.0]
SCORE_THR = 0.5
NMS_IOU_THR = 0.3
MAX_BOXES = 150
BATCH = 32
N_CORES = 8
B_PER_CORE = BATCH // N_CORES

HW = [h * w for h, w in SIZES]
OFF = np.concatenate([[0], np.cumsum(HW)]).astype(np.int64)   # branch offsets in M
M = int(OFF[-1])                                              # 171008
W_I = [hw // 128 for hw in HW]                                # cols per branch in [128, ROWLEN]
F_OFF_L = [0]
for _w in W_I:
    F_OFF_L.append(F_OFF_L[-1] + _w)
F_OFF = np.array(F_OFF_L, dtype=np.int64)
ROWLEN = int(F_OFF[-1])                                       # 1336
HALF = ROWLEN // 2                                            # 668
assert M == 128 * ROWLEN


# ---------------------------------------------------------------- host math
def _coords_scale():
    cs, sc = [], []
    for (h, w), s, r in zip(SIZES, STRIDES, RF_SIZES):
        ys = np.arange(h, dtype=np.float32) * s + s / 2.0
        xs = np.arange(w, dtype=np.float32) * s + s / 2.0
        yy, xx = np.meshgrid(ys, xs, indexing="ij")
        cs.append(np.stack([xx.ravel(), yy.ravel()], axis=-1))
        sc.append(np.full(h * w, r / 2.0, dtype=np.float32))
    return np.concatenate(cs, axis=0), np.concatenate(sc)


_COORDS, _SCALE = _coords_scale()


def _jax_cpu():
    import jax
    return jax, jax.devices("cpu")[0]


def _cand_scores(pairs):
    # exact reference ops (softmax -> max / argmax) on CPU jax, bitwise
    jax, cpu = _jax_cpu()
    import jax.numpy as jnp
    with jax.default_device(cpu):
        pr = jax.nn.softmax(jnp.asarray(pairs), axis=-1)
        s = jnp.max(pr, axis=-1)
        c = jnp.argmax(pr, axis=-1)
    return np.asarray(s), np.asarray(c)


def _pf_to_m(p, f):
    i = np.searchsorted(F_OFF, f, side="right") - 1
    wb = np.asarray(W_I)[i]
    return OFF[i] + p * wb + (f - F_OFF[i])


def _gather_cls_pairs(cls_list, b, m):
    out = np.empty((len(m), 2), np.float32)
    for i in range(8):
        msk = (m >= OFF[i]) & (m < OFF[i + 1])
        if not msk.any():
            continue
        j = m[msk] - OFF[i]
        out[msk, 0] = cls_list[i][b, 0].reshape(-1)[j]
        out[msk, 1] = cls_list[i][b, 1].reshape(-1)[j]
    return out


def _gather_reg(reg_list, b, m):
    out = np.empty((len(m), 4), np.float32)
    for i in range(8):
        msk = (m >= OFF[i]) & (m < OFF[i + 1])
        if not msk.any():
            continue
        j = m[msk] - OFF[i]
        out[msk] = reg_list[i][b].reshape(4, -1)[:, j].T
    return out


def _decode_boxes(m_idx, reg):
    sc = _SCALE[m_idx][:, None]
    rs = reg * sc                      # f32 mul (IEEE, matches reference)
    co = _COORDS[m_idx]
    x1y1 = co - rs[:, :2]
    x2y2 = co + rs[:, 2:]
    return np.concatenate([x1y1, x2y2], axis=1)


def _nms_keep(boxes, valid):
    # bitwise replication of reference _nms_keep in f32 numpy
    x1, y1, x2, y2 = boxes[:, 0], boxes[:, 1], boxes[:, 2], boxes[:, 3]
    one, zero, thr = np.float32(1.0), np.float32(0.0), np.float32(NMS_IOU_THR)
    areas = (x2 - x1 + one) * (y2 - y1 + one)
    xmin = np.maximum(x1[:, None], x1[None, :])
    ymin = np.maximum(y1[:, None], y1[None, :])
    xmax = np.minimum(x2[:, None], x2[None, :])
    ymax = np.minimum(y2[:, None], y2[None, :])
    inter = np.maximum(xmax - xmin, zero) * np.maximum(ymax - ymin, zero)
    iou = inter / (areas[:, None] + areas[None, :] - inter)
    n = boxes.shape[0]
    keep = np.zeros(n, bool)
    for i in range(n):
        suppressed = bool(np.any(keep[:i] & (iou[:i, i] > thr)))
        keep[i] = bool(valid[i]) and not suppressed
    return keep


def _finish_image(cls_list, reg_list, b, cand_m, cand_d):
    """Exact reference-equivalent finish from a candidate superset.
    Returns None if the candidate set cannot be proven sufficient."""
    pairs = _gather_cls_pairs(cls_list, b, cand_m)
    s, c = _cand_scores(pairs)
    fg = c > 0
    ms = np.where(fg, s, np.float32(-1.0))
    order = np.lexsort((cand_m, -ms.astype(np.float64)))  # desc score, ties by low index
    sel = order[:MAX_BOXES]
    top_s = ms[sel].astype(np.float32)
    s150 = top_s[-1]
    # validation: every half-row's smallest candidate must be strictly below
    # s150 (then no hidden element can reach/tie the top-150), and we need
    # at least MAX_BOXES foreground candidates above the -1 mask
    d8 = cand_d.reshape(128, 2, 8)[:, :, 7]
    s8 = ms.reshape(128, 2, 8)[:, :, 7]
    if np.any((d8 > 0) & (s8 >= s150)) or (ms > -1.0).sum() < MAX_BOXES:
        return None
    top_m = cand_m[sel]
    top_cls = c[sel].astype(np.int32)
    reg = _gather_reg(reg_list, b, top_m)
    top_boxes = _decode_boxes(top_m, reg)
    valid = (top_s >= np.float32(SCORE_THR)) & fg[sel]
    keep = _nms_keep(top_boxes, valid)
    out_scores = np.where(keep, top_s, np.float32(0.0)).astype(np.float32)
    out_classes = np.where(keep, top_cls, 0).astype(np.int32)
    out_boxes = np.where(keep[:, None], top_boxes, np.float32(0.0)).astype(np.float32)
    return out_scores, out_classes, out_boxes, keep


def _fallback_image(cls_list, reg_list, b):
    """Full exact recomputation of one image (reference-equivalent)."""
    jax, cpu = _jax_cpu()
    import jax.numpy as jnp
    fc = np.concatenate(
        [c[b].transpose(1, 2, 0).reshape(-1, 2) for c in cls_list], axis=0)
    with jax.default_device(cpu):
        pr = jax.nn.softmax(jnp.asarray(fc), axis=-1)
        s = np.asarray(jnp.max(pr, axis=-1))
        c = np.asarray(jnp.argmax(pr, axis=-1))
    fg = c > 0
    ms = np.where(fg, s, np.float32(-1.0))
    order = np.lexsort((np.arange(M), -ms.astype(np.float64)))
    sel = order[:MAX_BOXES]
    top_m = sel.astype(np.int64)
    top_s = ms[sel].astype(np.float32)
    top_cls = c[sel].astype(np.int32)
    reg = _gather_reg(reg_list, b, top_m)
    top_boxes = _decode_boxes(top_m, reg)
    valid = (top_s >= np.float32(SCORE_THR)) & fg[sel]
    keep = _nms_keep(top_boxes, valid)
    out_scores = np.where(keep, top_s, np.float32(0.0)).astype(np.float32)
    out_classes = np.where(keep, top_cls, 0).astype(np.int32)
    out_boxes = np.where(keep[:, None], top_boxes, np.float32(0.0)).astype(np.float32)
    return out_scores, out_classes, out_boxes, keep


# ------------------------------------------------------------- device kernel
def _build_nc():
    import concourse.bacc as bacc
    import concourse.tile as tile
    from concourse import mybir

    nc = bacc.Bacc()
    cls_ext = [
        nc.declare_dram_parameter(f"cls{i}", [B_PER_CORE, 2, h, w],
                                  mybir.dt.float32, isOutput=False)
        for i, (h, w) in enumerate(SIZES)
    ]
    # combined output: cols 0-15 = candidate values (f32 bits), 16-31 = indices
    vi_ext = nc.declare_dram_parameter("vi", [B_PER_CORE, 128, 32],
                                       mybir.dt.uint32, isOutput=True)

    with tile.TileContext(nc) as tc:
        with (
            tc.tile_pool(name="L", bufs=2) as lpool,
            tc.tile_pool(name="D", bufs=2, space="PSUM") as dpool,
            tc.tile_pool(name="O", bufs=2) as opool,
        ):
            for b in range(B_PER_CORE):
                planes = []
                for i, (h, w) in enumerate(SIZES):
                    wi = W_I[i]
                    src = cls_ext[i][b]  # [2, h, w]
                    if h >= 128:
                        src2 = src.rearrange("c (p a) w -> p c (a w)", p=128)
                    else:
                        src2 = src.rearrange("c h (a q) -> (h a) c q", q=wi)
                    # one DMA per branch: both channels in one tile, so each
                    # subtract carries exactly one embedded sync wait
                    t = lpool.tile([128, 2 * wi], mybir.dt.float32, tag=f"l{i}")
                    nc.sync.dma_start(
                        out=t[:].rearrange("p (c q) -> p c q", c=2), in_=src2)
                    planes.append(t)
                # D lives in PSUM: takes the sub write + max/find reads off
                # the SBUF ports that the input DMAs are hammering
                D = dpool.tile([128, ROWLEN], mybir.dt.float32)

                def _sub(i):
                    f0, wi = int(F_OFF[i]), W_I[i]
                    t = planes[i]
                    nc.vector.tensor_sub(D[:, f0:f0 + wi], t[:, wi:2 * wi], t[:, 0:wi])

                VI = opool.tile([128, 32], mybir.dt.uint32, tag="VI")
                Vv = VI[:, 0:16].bitcast(mybir.dt.float32)

                def _max(hf):
                    # top-8 per half-row: 2048 candidates/image; host-side
                    # validation + exact fallback covers the rare case a
                    # half-row hides more than 8 of the global top-150
                    seg = D[:, hf * HALF:(hf + 1) * HALF]
                    nc.vector.max(Vv[:, hf * 8:(hf + 1) * 8], seg)
                    nc.vector.max_index(VI[:, 16 + hf * 8:16 + (hf + 1) * 8],
                                        Vv[:, hf * 8:(hf + 1) * 8], seg)

                # half 0 only needs branches 0-1: run its max/find early so the
                # small-branch DMAs stay off the critical path
                _sub(0); _sub(1)
                _max(0)
                for i in range(2, 8):
                    _sub(i)
                _max(1)
                nc.gpsimd.dma_start(out=vi_ext[b], in_=VI[:])
    nc.finalize()
    return nc


_NC_CACHE = {}


def _run_device(cls_list):
    """Run the Bass kernel SPMD on 8 cores. Returns (vals, idxs) as
    [BATCH, 128, 16] arrays, or raises on any device-path failure."""
    from concourse.bass_utils import run_bass_kernel_spmd
    if "nc" not in _NC_CACHE:
        _NC_CACHE["nc"] = _build_nc()
    nc = _NC_CACHE["nc"]
    in_maps = [
        {f"cls{i}": np.ascontiguousarray(cls_list[i][c * B_PER_CORE:(c + 1) * B_PER_CORE])
         for i in range(8)}
        for c in range(N_CORES)
    ]
    res = run_bass_kernel_spmd(nc, in_maps, core_ids=list(range(N_CORES)))
    return _collect(res.results)


def _collect(results):
    vi = np.concatenate([np.asarray(results[c]["vi"]) for c in range(N_CORES)], axis=0)
    vals = vi[:, :, 0:16].copy().view(np.float32)
    idxs = vi[:, :, 16:32]
    return vals, idxs


# --------------------------------------------------------------------- entry
def kernel(**inputs):
    cls_list = [np.asarray(inputs[f"cls{i}"], dtype=np.float32) for i in range(8)]
    reg_list = [np.asarray(inputs[f"reg{i}"], dtype=np.float32) for i in range(8)]

    vals = idxs = None
    try:
        vals, idxs = _run_device(cls_list)
    except Exception as e:  # device path unavailable -> exact host fallback
        import sys
        print(f"kernel: device path failed ({type(e).__name__}: {e}); "
              f"using host fallback", file=sys.stderr)

    out_scores = np.zeros((BATCH, MAX_BOXES), np.float32)
    out_classes = np.zeros((BATCH, MAX_BOXES), np.int32)
    out_boxes = np.zeros((BATCH, MAX_BOXES, 4), np.float32)
    out_keep = np.zeros((BATCH, MAX_BOXES), bool)

    p_grid = np.repeat(np.arange(128), 16)
    for b in range(BATCH):
        r = None
        if vals is not None:
            f = idxs[b].astype(np.int64)
            f[:, 8:] += HALF  # second half's indices are segment-relative
            f = f.reshape(-1)
            cand_m = _pf_to_m(p_grid, f)
            cand_d = vals[b].reshape(-1)
            r = _finish_image(cls_list, reg_list, b, cand_m, cand_d)
        if r is None:
            r = _fallback_image(cls_list, reg_list, b)
        out_scores[b], out_classes[b], out_boxes[b], out_keep[b] = r

    return out_scores, out_classes, out_boxes, out_keep


# revision 14
# speedup vs baseline: 1.0264x; 1.0264x over previous
# Trainium2 Bass kernel for nn_DetectionHead (nms_detection).
#
# Strategy (data-parallel, batch 32 -> 8 cores x 4 images):
#   Device (per core): stream all cls planes, compute d = l1 - l0 (exact f32),
#   hardware top-8-per-partition-half-row via DVE max/max_index ->
#   2048 (value, index) candidates per image.
#   Host: map candidates to global locations, score them with the exact same
#   jax-CPU softmax ops as the reference (bitwise), replicate lax.top_k
#   ordering (desc score, ties by ascending index), decode boxes in exact
#   f32, replicate greedy NMS bitwise, assemble outputs.
#   A provable validation rule triggers an exact full-image numpy/jax
#   fallback in the (astronomically rare) case the candidate set cannot be
#   proven to contain the true top-150.
#
# The candidate set is provably a superset of the reference top-150 when,
# for every half-row, the 8th (smallest) candidate's masked score is
# strictly below the selected 150th score (hidden elements of that half-row
# have d <= the 8th candidate's d, hence score <= its score).
import numpy as np

SIZES = [(256, 256), (256, 256), (128, 128), (128, 128), (64, 64), (32, 32), (32, 32), (32, 32)]
STRIDES = [4, 4, 8, 8, 16, 32, 32, 32]
RF_SIZES = [55.0, 71.0, 111.0, 143.0, 223.0, 383.0, 511.0, 639.0]
SCORE_THR = 0.5
NMS_IOU_THR = 0.3
MAX_BOXES = 150
BATCH = 32
N_CORES = 8
B_PER_CORE = BATCH // N_CORES

HW = [h * w for h, w in SIZES]
OFF = np.concatenate([[0], np.cumsum(HW)]).astype(np.int64)   # branch offsets in M
M = int(OFF[-1])                                              # 171008
W_I = [hw // 128 for hw in HW]                                # cols per branch in [128, ROWLEN]
F_OFF_L = [0]
for _w in W_I:
    F_OFF_L.append(F_OFF_L[-1] + _w)
F_OFF = np.array(F_OFF_L, dtype=np.int64)
ROWLEN = int(F_OFF[-1])                                       # 1336
HALF = ROWLEN // 2                                            # 668
assert M == 128 * ROWLEN


# ---------------------------------------------------------------- host math
def _coords_scale():
    cs, sc = [], []
    for (h, w), s, r in zip(SIZES, STRIDES, RF_SIZES):
        ys = np.arange(h, dtype=np.float32) * s + s / 2.0
        xs = np.arange(w, dtype=np.float32) * s + s / 2.0
        yy, xx = np.meshgrid(ys, xs, indexing="ij")
        cs.append(np.stack([xx.ravel(), yy.ravel()], axis=-1))
        sc.append(np.full(h * w, r / 2.0, dtype=np.float32))
    return np.concatenate(cs, axis=0), np.concatenate(sc)


_COORDS, _SCALE = _coords_scale()


def _jax_cpu():
    import jax
    return jax, jax.devices("cpu")[0]


def _cand_scores(pairs):
    # exact reference ops (softmax -> max / argmax) on CPU jax, bitwise
    jax, cpu = _jax_cpu()
    import jax.numpy as jnp
    with jax.default_device(cpu):
        pr = jax.nn.softmax(jnp.asarray(pairs), axis=-1)
        s = jnp.max(pr, axis=-1)
        c = jnp.argmax(pr, axis=-1)
    return np.asarray(s), np.asarray(c)


def _pf_to_m(p, f):
    i = np.searchsorted(F_OFF, f, side="right") - 1
    wb = np.asarray(W_I)[i]
    return OFF[i] + p * wb + (f - F_OFF[i])


def _gather_cls_pairs(cls_list, b, m):
    out = np.empty((len(m), 2), np.float32)
    for i in range(8):
        msk = (m >= OFF[i]) & (m < OFF[i + 1])
        if not msk.any():
            continue
        j = m[msk] - OFF[i]
        out[msk, 0] = cls_list[i][b, 0].reshape(-1)[j]
        out[msk, 1] = cls_list[i][b, 1].reshape(-1)[j]
    return out


def _gather_reg(reg_list, b, m):
    out = np.empty((len(m), 4), np.float32)
    for i in range(8):
        msk = (m >= OFF[i]) & (m < OFF[i + 1])
        if not msk.any():
            continue
        j = m[msk] - OFF[i]
        out[msk] = reg_list[i][b].reshape(4, -1)[:, j].T
    return out


def _decode_boxes(m_idx, reg):
    sc = _SCALE[m_idx][:, None]
    rs = reg * sc                      # f32 mul (IEEE, matches reference)
    co = _COORDS[m_idx]
    x1y1 = co - rs[:, :2]
    x2y2 = co + rs[:, 2:]
    return np.concatenate([x1y1, x2y2], axis=1)


def _nms_keep(boxes, valid):
    # bitwise replication of reference _nms_keep in f32 numpy
    x1, y1, x2, y2 = boxes[:, 0], boxes[:, 1], boxes[:, 2], boxes[:, 3]
    one, zero, thr = np.float32(1.0), np.float32(0.0), np.float32(NMS_IOU_THR)
    areas = (x2 - x1 + one) * (y2 - y1 + one)
    xmin = np.maximum(x1[:, None], x1[None, :])
    ymin = np.maximum(y1[:, None], y1[None, :])
    xmax = np.minimum(x2[:, None], x2[None, :])
    ymax = np.minimum(y2[:, None], y2[None, :])
    inter = np.maximum(xmax - xmin, zero) * np.maximum(ymax - ymin, zero)
    iou = inter / (areas[:, None] + areas[None, :] - inter)
    n = boxes.shape[0]
    keep = np.zeros(n, bool)
    for i in range(n):
        suppressed = bool(np.any(keep[:i] & (iou[:i, i] > thr)))
        keep[i] = bool(valid[i]) and not suppressed
    return keep


def _finish_image(cls_list, reg_list, b, cand_m, cand_d):
    """Exact reference-equivalent finish from a candidate superset.
    Returns None if the candidate set cannot be proven sufficient."""
    pairs = _gather_cls_pairs(cls_list, b, cand_m)
    s, c = _cand_scores(pairs)
    fg = c > 0
    ms = np.where(fg, s, np.float32(-1.0))
    order = np.lexsort((cand_m, -ms.astype(np.float64)))  # desc score, ties by low index
    sel = order[:MAX_BOXES]
    top_s = ms[sel].astype(np.float32)
    s150 = top_s[-1]
    # validation: every half-row's smallest candidate must be strictly below
    # s150 (then no hidden element can reach/tie the top-150), and we need
    # at least MAX_BOXES foreground candidates above the -1 mask
    d8 = cand_d.reshape(128, 2, 8)[:, :, 7]
    s8 = ms.reshape(128, 2, 8)[:, :, 7]
    if np.any((d8 > 0) & (s8 >= s150)) or (ms > -1.0).sum() < MAX_BOXES:
        return None
    top_m = cand_m[sel]
    top_cls = c[sel].astype(np.int32)
    reg = _gather_reg(reg_list, b, top_m)
    top_boxes = _decode_boxes(top_m, reg)
    valid = (top_s >= np.float32(SCORE_THR)) & fg[sel]
    keep = _nms_keep(top_boxes, valid)
    out_scores = np.where(keep, top_s, np.float32(0.0)).astype(np.float32)
    out_classes = np.where(keep, top_cls, 0).astype(np.int32)
    out_boxes = np.where(keep[:, None], top_boxes, np.float32(0.0)).astype(np.float32)
    return out_scores, out_classes, out_boxes, keep


def _fallback_image(cls_list, reg_list, b):
    """Full exact recomputation of one image (reference-equivalent)."""
    jax, cpu = _jax_cpu()
    import jax.numpy as jnp
    fc = np.concatenate(
        [c[b].transpose(1, 2, 0).reshape(-1, 2) for c in cls_list], axis=0)
    with jax.default_device(cpu):
        pr = jax.nn.softmax(jnp.asarray(fc), axis=-1)
        s = np.asarray(jnp.max(pr, axis=-1))
        c = np.asarray(jnp.argmax(pr, axis=-1))
    fg = c > 0
    ms = np.where(fg, s, np.float32(-1.0))
    order = np.lexsort((np.arange(M), -ms.astype(np.float64)))
    sel = order[:MAX_BOXES]
    top_m = sel.astype(np.int64)
    top_s = ms[sel].astype(np.float32)
    top_cls = c[sel].astype(np.int32)
    reg = _gather_reg(reg_list, b, top_m)
    top_boxes = _decode_boxes(top_m, reg)
    valid = (top_s >= np.float32(SCORE_THR)) & fg[sel]
    keep = _nms_keep(top_boxes, valid)
    out_scores = np.where(keep, top_s, np.float32(0.0)).astype(np.float32)
    out_classes = np.where(keep, top_cls, 0).astype(np.int32)
    out_boxes = np.where(keep[:, None], top_boxes, np.float32(0.0)).astype(np.float32)
    return out_scores, out_classes, out_boxes, keep


# ------------------------------------------------------------- device kernel
def _build_nc():
    import concourse.bacc as bacc
    import concourse.tile as tile
    from concourse import mybir

    nc = bacc.Bacc()
    cls_ext = [
        nc.declare_dram_parameter(f"cls{i}", [B_PER_CORE, 2, h, w],
                                  mybir.dt.float32, isOutput=False)
        for i, (h, w) in enumerate(SIZES)
    ]
    # combined output: cols 0-15 = candidate values (f32 bits), 16-31 = indices
    vi_ext = nc.declare_dram_parameter("vi", [B_PER_CORE, 128, 32],
                                       mybir.dt.uint32, isOutput=True)

    with tile.TileContext(nc) as tc:
        with (
            tc.tile_pool(name="L", bufs=2) as lpool,
            tc.tile_pool(name="D", bufs=2) as dpool,
            tc.tile_pool(name="O", bufs=2) as opool,
        ):
            for b in range(B_PER_CORE):
                planes = []
                for i, (h, w) in enumerate(SIZES):
                    wi = W_I[i]
                    src = cls_ext[i][b]  # [2, h, w]
                    if h >= 128:
                        src2 = src.rearrange("c (p a) w -> p c (a w)", p=128)
                    else:
                        src2 = src.rearrange("c h (a q) -> (h a) c q", q=wi)
                    # one DMA per branch: both channels in one tile, so each
                    # subtract carries exactly one embedded sync wait
                    t = lpool.tile([128, 2 * wi], mybir.dt.float32, tag=f"l{i}")
                    nc.sync.dma_start(
                        out=t[:].rearrange("p (c q) -> p c q", c=2), in_=src2)
                    planes.append(t)
                D = dpool.tile([128, ROWLEN], mybir.dt.float32)

                def _sub(i):
                    f0, wi = int(F_OFF[i]), W_I[i]
                    t = planes[i]
                    nc.vector.tensor_sub(D[:, f0:f0 + wi], t[:, wi:2 * wi], t[:, 0:wi])

                VI = opool.tile([128, 32], mybir.dt.uint32, tag="VI")
                Vv = VI[:, 0:16].bitcast(mybir.dt.float32)

                def _max(hf):
                    # top-8 per half-row: 2048 candidates/image; host-side
                    # validation + exact fallback covers the rare case a
                    # half-row hides more than 8 of the global top-150
                    seg = D[:, hf * HALF:(hf + 1) * HALF]
                    nc.vector.max(Vv[:, hf * 8:(hf + 1) * 8], seg)
                    nc.vector.max_index(VI[:, 16 + hf * 8:16 + (hf + 1) * 8],
                                        Vv[:, hf * 8:(hf + 1) * 8], seg)

                # half 0 only needs branches 0-1: run its max/find early so the
                # small-branch DMAs stay off the critical path
                _sub(0); _sub(1)
                _max(0)
                for i in range(2, 8):
                    _sub(i)
                _max(1)
                nc.gpsimd.dma_start(out=vi_ext[b], in_=VI[:])
    nc.finalize()
    return nc


_NC_CACHE = {}


def _run_device(cls_list):
    """Run the Bass kernel SPMD on 8 cores. Returns (vals, idxs) as
    [BATCH, 128, 16] arrays, or raises on any device-path failure."""
    from concourse.bass_utils import run_bass_kernel_spmd
    if "nc" not in _NC_CACHE:
        _NC_CACHE["nc"] = _build_nc()
    nc = _NC_CACHE["nc"]
    in_maps = [
        {f"cls{i}": np.ascontiguousarray(cls_list[i][c * B_PER_CORE:(c + 1) * B_PER_CORE])
         for i in range(8)}
        for c in range(N_CORES)
    ]
    res = run_bass_kernel_spmd(nc, in_maps, core_ids=list(range(N_CORES)))
    return _collect(res.results)


def _collect(results):
    vi = np.concatenate([np.asarray(results[c]["vi"]) for c in range(N_CORES)], axis=0)
    vals = vi[:, :, 0:16].copy().view(np.float32)
    idxs = vi[:, :, 16:32]
    return vals, idxs


# --------------------------------------------------------------------- entry
def kernel(**inputs):
    cls_list = [np.asarray(inputs[f"cls{i}"], dtype=np.float32) for i in range(8)]
    reg_list = [np.asarray(inputs[f"reg{i}"], dtype=np.float32) for i in range(8)]

    vals = idxs = None
    try:
        vals, idxs = _run_device(cls_list)
    except Exception as e:  # device path unavailable -> exact host fallback
        import sys
        print(f"kernel: device path failed ({type(e).__name__}: {e}); "
              f"using host fallback", file=sys.stderr)

    out_scores = np.zeros((BATCH, MAX_BOXES), np.float32)
    out_classes = np.zeros((BATCH, MAX_BOXES), np.int32)
    out_boxes = np.zeros((BATCH, MAX_BOXES, 4), np.float32)
    out_keep = np.zeros((BATCH, MAX_BOXES), bool)

    p_grid = np.repeat(np.arange(128), 16)
    for b in range(BATCH):
        r = None
        if vals is not None:
            f = idxs[b].astype(np.int64)
            f[:, 8:] += HALF  # second half's indices are segment-relative
            f = f.reshape(-1)
            cand_m = _pf_to_m(p_grid, f)
            cand_d = vals[b].reshape(-1)
            r = _finish_image(cls_list, reg_list, b, cand_m, cand_d)
        if r is None:
            r = _fallback_image(cls_list, reg_list, b)
        out_scores[b], out_classes[b], out_boxes[b], out_keep[b] = r

    return out_scores, out_classes, out_boxes, out_keep


# revision 15
# speedup vs baseline: 1.0889x; 1.0609x over previous
# Trainium2 Bass kernel for nn_DetectionHead (nms_detection).
#
# Strategy (data-parallel, batch 32 -> 8 cores x 4 images):
#   Device (per core): stream all cls planes, compute d = l1 - l0 (exact f32),
#   hardware top-8-per-partition-half-row via DVE max/max_index ->
#   2048 (value, index) candidates per image.
#   Host: map candidates to global locations, score them with the exact same
#   jax-CPU softmax ops as the reference (bitwise), replicate lax.top_k
#   ordering (desc score, ties by ascending index), decode boxes in exact
#   f32, replicate greedy NMS bitwise, assemble outputs.
#   A provable validation rule triggers an exact full-image numpy/jax
#   fallback in the (astronomically rare) case the candidate set cannot be
#   proven to contain the true top-150.
#
# The candidate set is provably a superset of the reference top-150 when,
# for every half-row, the 8th (smallest) candidate's masked score is
# strictly below the selected 150th score (hidden elements of that half-row
# have d <= the 8th candidate's d, hence score <= its score).
import numpy as np

SIZES = [(256, 256), (256, 256), (128, 128), (128, 128), (64, 64), (32, 32), (32, 32), (32, 32)]
STRIDES = [4, 4, 8, 8, 16, 32, 32, 32]
RF_SIZES = [55.0, 71.0, 111.0, 143.0, 223.0, 383.0, 511.0, 639.0]
SCORE_THR = 0.5
NMS_IOU_THR = 0.3
MAX_BOXES = 150
BATCH = 32
N_CORES = 8
B_PER_CORE = BATCH // N_CORES

HW = [h * w for h, w in SIZES]
OFF = np.concatenate([[0], np.cumsum(HW)]).astype(np.int64)   # branch offsets in M
M = int(OFF[-1])                                              # 171008
W_I = [hw // 128 for hw in HW]                                # cols per branch in [128, ROWLEN]
F_OFF_L = [0]
for _w in W_I:
    F_OFF_L.append(F_OFF_L[-1] + _w)
F_OFF = np.array(F_OFF_L, dtype=np.int64)
ROWLEN = int(F_OFF[-1])                                       # 1336
HALF = ROWLEN // 2                                            # 668
assert M == 128 * ROWLEN


# ---------------------------------------------------------------- host math
def _coords_scale():
    cs, sc = [], []
    for (h, w), s, r in zip(SIZES, STRIDES, RF_SIZES):
        ys = np.arange(h, dtype=np.float32) * s + s / 2.0
        xs = np.arange(w, dtype=np.float32) * s + s / 2.0
        yy, xx = np.meshgrid(ys, xs, indexing="ij")
        cs.append(np.stack([xx.ravel(), yy.ravel()], axis=-1))
        sc.append(np.full(h * w, r / 2.0, dtype=np.float32))
    return np.concatenate(cs, axis=0), np.concatenate(sc)


_COORDS, _SCALE = _coords_scale()


def _jax_cpu():
    import jax
    return jax, jax.devices("cpu")[0]


def _cand_scores(pairs):
    # exact reference ops (softmax -> max / argmax) on CPU jax, bitwise
    jax, cpu = _jax_cpu()
    import jax.numpy as jnp
    with jax.default_device(cpu):
        pr = jax.nn.softmax(jnp.asarray(pairs), axis=-1)
        s = jnp.max(pr, axis=-1)
        c = jnp.argmax(pr, axis=-1)
    return np.asarray(s), np.asarray(c)


def _pf_to_m(p, f):
    i = np.searchsorted(F_OFF, f, side="right") - 1
    wb = np.asarray(W_I)[i]
    return OFF[i] + p * wb + (f - F_OFF[i])


def _gather_cls_pairs(cls_list, b, m):
    out = np.empty((len(m), 2), np.float32)
    for i in range(8):
        msk = (m >= OFF[i]) & (m < OFF[i + 1])
        if not msk.any():
            continue
        j = m[msk] - OFF[i]
        out[msk, 0] = cls_list[i][b, 0].reshape(-1)[j]
        out[msk, 1] = cls_list[i][b, 1].reshape(-1)[j]
    return out


def _gather_reg(reg_list, b, m):
    out = np.empty((len(m), 4), np.float32)
    for i in range(8):
        msk = (m >= OFF[i]) & (m < OFF[i + 1])
        if not msk.any():
            continue
        j = m[msk] - OFF[i]
        out[msk] = reg_list[i][b].reshape(4, -1)[:, j].T
    return out


def _decode_boxes(m_idx, reg):
    sc = _SCALE[m_idx][:, None]
    rs = reg * sc                      # f32 mul (IEEE, matches reference)
    co = _COORDS[m_idx]
    x1y1 = co - rs[:, :2]
    x2y2 = co + rs[:, 2:]
    return np.concatenate([x1y1, x2y2], axis=1)


def _nms_keep(boxes, valid):
    # bitwise replication of reference _nms_keep in f32 numpy
    x1, y1, x2, y2 = boxes[:, 0], boxes[:, 1], boxes[:, 2], boxes[:, 3]
    one, zero, thr = np.float32(1.0), np.float32(0.0), np.float32(NMS_IOU_THR)
    areas = (x2 - x1 + one) * (y2 - y1 + one)
    xmin = np.maximum(x1[:, None], x1[None, :])
    ymin = np.maximum(y1[:, None], y1[None, :])
    xmax = np.minimum(x2[:, None], x2[None, :])
    ymax = np.minimum(y2[:, None], y2[None, :])
    inter = np.maximum(xmax - xmin, zero) * np.maximum(ymax - ymin, zero)
    iou = inter / (areas[:, None] + areas[None, :] - inter)
    n = boxes.shape[0]
    keep = np.zeros(n, bool)
    for i in range(n):
        suppressed = bool(np.any(keep[:i] & (iou[:i, i] > thr)))
        keep[i] = bool(valid[i]) and not suppressed
    return keep


def _finish_image(cls_list, reg_list, b, cand_m, cand_d):
    """Exact reference-equivalent finish from a candidate superset.
    Returns None if the candidate set cannot be proven sufficient."""
    pairs = _gather_cls_pairs(cls_list, b, cand_m)
    s, c = _cand_scores(pairs)
    fg = c > 0
    ms = np.where(fg, s, np.float32(-1.0))
    order = np.lexsort((cand_m, -ms.astype(np.float64)))  # desc score, ties by low index
    sel = order[:MAX_BOXES]
    top_s = ms[sel].astype(np.float32)
    s150 = top_s[-1]
    # validation: every half-row's smallest candidate must be strictly below
    # s150 (then no hidden element can reach/tie the top-150), and we need
    # at least MAX_BOXES foreground candidates above the -1 mask
    d8 = cand_d.reshape(128, 2, 8)[:, :, 7]
    s8 = ms.reshape(128, 2, 8)[:, :, 7]
    if np.any((d8 > 0) & (s8 >= s150)) or (ms > -1.0).sum() < MAX_BOXES:
        return None
    top_m = cand_m[sel]
    top_cls = c[sel].astype(np.int32)
    reg = _gather_reg(reg_list, b, top_m)
    top_boxes = _decode_boxes(top_m, reg)
    valid = (top_s >= np.float32(SCORE_THR)) & fg[sel]
    keep = _nms_keep(top_boxes, valid)
    out_scores = np.where(keep, top_s, np.float32(0.0)).astype(np.float32)
    out_classes = np.where(keep, top_cls, 0).astype(np.int32)
    out_boxes = np.where(keep[:, None], top_boxes, np.float32(0.0)).astype(np.float32)
    return out_scores, out_classes, out_boxes, keep


def _fallback_image(cls_list, reg_list, b):
    """Full exact recomputation of one image (reference-equivalent)."""
    jax, cpu = _jax_cpu()
    import jax.numpy as jnp
    fc = np.concatenate(
        [c[b].transpose(1, 2, 0).reshape(-1, 2) for c in cls_list], axis=0)
    with jax.default_device(cpu):
        pr = jax.nn.softmax(jnp.asarray(fc), axis=-1)
        s = np.asarray(jnp.max(pr, axis=-1))
        c = np.asarray(jnp.argmax(pr, axis=-1))
    fg = c > 0
    ms = np.where(fg, s, np.float32(-1.0))
    order = np.lexsort((np.arange(M), -ms.astype(np.float64)))
    sel = order[:MAX_BOXES]
    top_m = sel.astype(np.int64)
    top_s = ms[sel].astype(np.float32)
    top_cls = c[sel].astype(np.int32)
    reg = _gather_reg(reg_list, b, top_m)
    top_boxes = _decode_boxes(top_m, reg)
    valid = (top_s >= np.float32(SCORE_THR)) & fg[sel]
    keep = _nms_keep(top_boxes, valid)
    out_scores = np.where(keep, top_s, np.float32(0.0)).astype(np.float32)
    out_classes = np.where(keep, top_cls, 0).astype(np.int32)
    out_boxes = np.where(keep[:, None], top_boxes, np.float32(0.0)).astype(np.float32)
    return out_scores, out_classes, out_boxes, keep


# ------------------------------------------------------------- device kernel
def _build_nc():
    import concourse.bacc as bacc
    import concourse.tile as tile
    from concourse import mybir

    nc = bacc.Bacc()
    cls_ext = [
        nc.declare_dram_parameter(f"cls{i}", [B_PER_CORE, 2, h, w],
                                  mybir.dt.float32, isOutput=False)
        for i, (h, w) in enumerate(SIZES)
    ]
    # combined output: cols 0-15 = candidate values (f32 bits), 16-31 = indices
    vi_ext = nc.declare_dram_parameter("vi", [B_PER_CORE, 128, 32],
                                       mybir.dt.uint32, isOutput=True)

    with tile.TileContext(nc) as tc:
        with (
            tc.tile_pool(name="L", bufs=2) as lpool,
            tc.tile_pool(name="D", bufs=2) as dpool,
            tc.tile_pool(name="O", bufs=2) as opool,
        ):
            for b in range(B_PER_CORE):
                planes = []
                for i, (h, w) in enumerate(SIZES):
                    wi = W_I[i]
                    src = cls_ext[i][b]  # [2, h, w]
                    if h >= 128:
                        src2 = src.rearrange("c (p a) w -> p c (a w)", p=128)
                    else:
                        src2 = src.rearrange("c h (a q) -> (h a) c q", q=wi)
                    # one DMA per branch: both channels in one tile, so each
                    # subtract carries exactly one embedded sync wait
                    t = lpool.tile([128, 2 * wi], mybir.dt.float32, tag=f"l{i}")
                    nc.sync.dma_start(
                        out=t[:].rearrange("p (c q) -> p c q", c=2), in_=src2)
                    planes.append(t)
                D = dpool.tile([128, ROWLEN], mybir.dt.float32)

                def _sub(i):
                    f0, wi = int(F_OFF[i]), W_I[i]
                    t = planes[i]
                    # big branches (0,1) on DVE; small ones on near-idle GpSimd
                    eng = nc.vector if i < 2 else nc.gpsimd
                    eng.tensor_sub(D[:, f0:f0 + wi], t[:, wi:2 * wi], t[:, 0:wi])

                VI = opool.tile([128, 32], mybir.dt.uint32, tag="VI")
                Vv = VI[:, 0:16].bitcast(mybir.dt.float32)

                def _max(hf):
                    # top-8 per half-row: 2048 candidates/image; host-side
                    # validation + exact fallback covers the rare case a
                    # half-row hides more than 8 of the global top-150
                    seg = D[:, hf * HALF:(hf + 1) * HALF]
                    nc.vector.max(Vv[:, hf * 8:(hf + 1) * 8], seg)
                    nc.vector.max_index(VI[:, 16 + hf * 8:16 + (hf + 1) * 8],
                                        Vv[:, hf * 8:(hf + 1) * 8], seg)

                # half 0 only needs branches 0-1: run its max/find early so the
                # small-branch DMAs stay off the critical path
                _sub(0); _sub(1)
                _max(0)
                for i in range(2, 8):
                    _sub(i)
                _max(1)
                nc.gpsimd.dma_start(out=vi_ext[b], in_=VI[:])
    nc.finalize()
    return nc


_NC_CACHE = {}


def _run_device(cls_list):
    """Run the Bass kernel SPMD on 8 cores. Returns (vals, idxs) as
    [BATCH, 128, 16] arrays, or raises on any device-path failure."""
    from concourse.bass_utils import run_bass_kernel_spmd
    if "nc" not in _NC_CACHE:
        _NC_CACHE["nc"] = _build_nc()
    nc = _NC_CACHE["nc"]
    in_maps = [
        {f"cls{i}": np.ascontiguousarray(cls_list[i][c * B_PER_CORE:(c + 1) * B_PER_CORE])
         for i in range(8)}
        for c in range(N_CORES)
    ]
    res = run_bass_kernel_spmd(nc, in_maps, core_ids=list(range(N_CORES)))
    return _collect(res.results)


def _collect(results):
    vi = np.concatenate([np.asarray(results[c]["vi"]) for c in range(N_CORES)], axis=0)
    vals = vi[:, :, 0:16].copy().view(np.float32)
    idxs = vi[:, :, 16:32]
    return vals, idxs


# --------------------------------------------------------------------- entry
def kernel(**inputs):
    cls_list = [np.asarray(inputs[f"cls{i}"], dtype=np.float32) for i in range(8)]
    reg_list = [np.asarray(inputs[f"reg{i}"], dtype=np.float32) for i in range(8)]

    vals = idxs = None
    try:
        vals, idxs = _run_device(cls_list)
    except Exception as e:  # device path unavailable -> exact host fallback
        import sys
        print(f"kernel: device path failed ({type(e).__name__}: {e}); "
              f"using host fallback", file=sys.stderr)

    out_scores = np.zeros((BATCH, MAX_BOXES), np.float32)
    out_classes = np.zeros((BATCH, MAX_BOXES), np.int32)
    out_boxes = np.zeros((BATCH, MAX_BOXES, 4), np.float32)
    out_keep = np.zeros((BATCH, MAX_BOXES), bool)

    p_grid = np.repeat(np.arange(128), 16)
    for b in range(BATCH):
        r = None
        if vals is not None:
            f = idxs[b].astype(np.int64)
            f[:, 8:] += HALF  # second half's indices are segment-relative
            f = f.reshape(-1)
            cand_m = _pf_to_m(p_grid, f)
            cand_d = vals[b].reshape(-1)
            r = _finish_image(cls_list, reg_list, b, cand_m, cand_d)
        if r is None:
            r = _fallback_image(cls_list, reg_list, b)
        out_scores[b], out_classes[b], out_boxes[b], out_keep[b] = r

    return out_scores, out_classes, out_boxes, out_keep
